# revision 35
# baseline (speedup 1.0000x reference)
"""Policy-masked multi-head attention block (ViT-style) on 8 TRN2 NeuronCores.

Sharding: data-parallel over batch. B=16 -> 2 batches per core, no collectives.

Math (per batch, matches reference up to O(1e-7)):
  qkv = x @ Wqkv + bqkv ; q,k,v per head (Dh=64)
  E[j,q] = exp(SCALE*z[j,q] + mb_j)   (z = k_j . q_q raw logits; mb_j = -1e4 for
                                       dropped/padded keys, 0 otherwise)
  E[q,q] = exp(SCALE*z[q,q])          (diagonal always kept)
  num[d,q] = sum_j v[j,d]*E[j,q] + (EPS/N)*sum_j v[j,d]
  den[q]   = sum_j E[j,q] + EPS
  attn_out[q,d] = num[d,q]/den[q] ;  out = attn_out @ Wproj + bproj

No max-subtraction: |SCALE*z| is O(1) for this input distribution, and the
reference's max-shift cancels in the ratio except for EPS terms (~1e-8 rel).

Layout: attention is computed keys-on-partitions / queries-on-free so that
 - the policy mask is a per-partition activation bias (free on ACT),
 - the key-sum (softmax denominator) rides the AV matmul as a ones column of V,
 - E is consumed directly as the moving operand of the AV matmul (no P matrix
   transpose).
"""

import os
import ml_dtypes
import numpy as np

import concourse.bass as bass
import concourse.bacc as bacc
import concourse.mybir as mybir
import concourse.tile as tile
from concourse.masks import make_identity

# problem constants (hardcoded per contract)
B = 16
N = 577
C = 768
H = 12
DH = 64
SCALE = DH ** -0.5
EPS = 1e-6
NCORES = 8
BB = B // NCORES          # batches per core
NPAD = 640                # tokens padded to 5*128
NT = NPAD // 128          # 5 token tiles
NF = C // 128             # 6 feature tiles
F32 = mybir.dt.float32
F32R = mybir.dt.float32r
BF16 = mybir.dt.bfloat16
MMDT_NAME = os.environ.get("KERNEL_MMDT", "bf16")
DT = BF16 if MMDT_NAME == "bf16" else F32R

MASK_NEG = -1.0e4
NQ1 = 320              # query region sizes: [0:320), [320:577)
NQ2 = N - 320          # 257


DEBUG = bool(int(os.environ.get("KERNEL_DEBUG", "0")))


def build_kernel():
    nc = bacc.Bacc()
    dbg = {}
    if DEBUG:
        dbg["qk"] = nc.declare_dram_parameter("dbg_qk", [128, 12, NPAD], F32, isOutput=True)
        dbg["v"] = nc.declare_dram_parameter("dbg_v", [128, NT, H, DH + 1], F32, isOutput=True)
        dbg["dt"] = nc.declare_dram_parameter("dbg_dt", [128, NT, H], F32, isOutput=True)
        dbg["e"] = nc.declare_dram_parameter("dbg_e", [NT, 128, NPAD], F32, isOutput=True)
        dbg["num"] = nc.declare_dram_parameter("dbg_num", [DH + 1, NPAD], F32, isOutput=True)
        dbg["attn"] = nc.declare_dram_parameter("dbg_attn", [128, NT, C], F32, isOutput=True)

    x_d = nc.declare_dram_parameter("x", [BB * NPAD, C], DT, isOutput=False)
    mb_d = nc.declare_dram_parameter("maskb", [BB, NPAD], F32, isOutput=False)
    wqkv_d = nc.declare_dram_parameter("wqkv", [C, 3 * C], DT, isOutput=False)
    wproj_d = nc.declare_dram_parameter("wproj", [C, C], DT, isOutput=False)
    bqkv_d = nc.declare_dram_parameter("bqkv", [3 * C], F32, isOutput=False)
    bproj_d = nc.declare_dram_parameter("bproj", [C], F32, isOutput=False)
    onehd_d = nc.declare_dram_parameter("ones_hd", [C, H], DT, isOutput=False)
    out_d = nc.declare_dram_parameter("out", [BB * N, C], F32, isOutput=True)

    with tile.TileContext(nc) as tc:
        with (
            tc.tile_pool(name="singles", bufs=1) as singles,
            tc.tile_pool(name="pbatch", bufs=1) as pb,
            tc.tile_pool(name="pqk", bufs=2) as pqk,
            tc.tile_pool(name="pe", bufs=3) as pe_pool,
            tc.tile_pool(name="pnum", bufs=2) as pnum,
            tc.tile_pool(name="psmall", bufs=4) as psmall,
            tc.tile_pool(name="ptrans", bufs=2) as ptrans,
            tc.tile_pool(name="pout", bufs=2) as pout,
            tc.tile_pool(name="ppA", bufs=2, space="PSUM") as ppA,
            tc.tile_pool(name="ppT", bufs=2, space="PSUM") as ppT,
            tc.tile_pool(name="ppN", bufs=1, space="PSUM") as ppN,
        ):
            # ---- constants ----
            wqkv_sb = singles.tile([128, NF, 3 * C], DT)
            wproj_sb = singles.tile([128, NF, C], DT)
            for f in range(NF):
                nc.scalar.dma_start(
                    wproj_sb[:, f, :],
                    wproj_d.rearrange("(f p) m -> f p m", p=128)[f],
                )
            bqkv_sb = singles.tile([128, 2 * NF], F32)  # q,k feature bias chunks
            nc.scalar.dma_start(
                bqkv_sb[:, :],
                bqkv_d[0 : 2 * C].rearrange("(m p) -> p m", p=128),
            )
            # v-bias and proj-bias broadcast across all 128 partitions
            bv_bc = singles.tile([128, C], F32)
            nc.gpsimd.dma_start(
                out=bv_bc[:, :], in_=bqkv_d[2 * C : 3 * C].partition_broadcast(128)
            )
            bproj_bc = singles.tile([128, C], F32)
            nc.gpsimd.dma_start(
                out=bproj_bc[:, :], in_=bproj_d[:].partition_broadcast(128)
            )
            onehd_sb = singles.tile([128, NF, H], DT)
            nc.scalar.dma_start(
                onehd_sb[:, :, :],
                onehd_d.rearrange("(f p) h -> p f h", p=128),
            )
            ident_f32 = singles.tile([128, 128], F32)
            make_identity(nc, ident_f32)
            ident = singles.tile([128, 128], DT)
            nc.vector.tensor_copy(ident[:, :], ident_f32[:, :])
            ones60 = singles.tile([128, NT * H], F32)
            nc.vector.memset(ones60, 1.0)
            for i in range(40):
                wps = ppT.tile([128, 128], F32, tag="ppT", name=f"warm{i}")
                nc.tensor.matmul(
                    wps[:, :], ident[:, :], ident[:, :], start=True, stop=True
                )

            for b in range(BB):
                # ---- load x (token-major chunks), transpose to xT ----
                mb_sb = psmall.tile([128, NT], F32, tag="mb_sb")
                nc.sync.dma_start(
                    mb_sb[:, :], mb_d[b].rearrange("(t p) -> p t", p=128)
                )
                xT = pb.tile([128, NF, NPAD], DT, tag="xT")
                for f in range(NF):
                    nc.sync.dma_start_transpose(
                        xT[:, f, :],
                        x_d[b * NPAD : (b + 1) * NPAD,
                            f * 128 : (f + 1) * 128],
                    )
                    if b == 0:
                        nc.sync.dma_start(
                            wqkv_sb[:, f, :],
                            wqkv_d.rearrange("(f p) m -> f p m", p=128)[f],
                        )

                # ---- QKV: q,k sections feature-major ----
                # qk_sb[p, m, :]: m in 0..5 -> q features, 6..11 -> k features
                qk_sb = pqk.tile([128, 2 * NF, NPAD], DT, tag="qk_sb")
                for m in range(2 * NF):
                    ps = ppA.tile([128, 2, 512], F32, tag="ppA")
                    for f in range(NF):
                        for s in range(2):
                            nc.tensor.matmul(
                                ps[:, s, 0:320],
                                wqkv_sb[:, f, m * 128 : (m + 1) * 128],
                                xT[:, f, s * 320 : s * 320 + 320],
                                start=(f == 0),
                                stop=(f == NF - 1),
                            )
                    nc.vector.tensor_scalar_add(
                        qk_sb[:, m, :].rearrange("p (s q) -> p s q", s=2),
                        ps[:, :, 0:320],
                        bqkv_sb[:, m : m + 1],
                    )

                if DEBUG and b == 0:
                    nc.sync.dma_start(dbg["qk"][:], qk_sb[:, :, :])

                # ---- QKV: v section token-major, per-head layout with ones col
                v_sb = pb.tile([128, NT, H, DH + 1], DT, tag="v_sb")
                for t in range(NT):
                    ps = ppA.tile([128, 2, 512], F32, tag="ppA")
                    for f in range(NF):
                        for n0, n1 in ((0, 512), (512, 768)):
                            nc.tensor.matmul(
                                ps.rearrange("p s q -> p (s q)")[:, n0:n1],
                                xT[:, f, t * 128 : (t + 1) * 128],
                                wqkv_sb[:, f, 2 * C + n0 : 2 * C + n1],
                                start=(f == 0),
                                stop=(f == NF - 1),
                            )
                    nc.vector.tensor_tensor(
                        v_sb[:, t, :, 0:DH],
                        ps.rearrange("p s q -> p (s q)")[:, 0:C].rearrange(
                            "p (h d) -> p h d", h=H
                        ),
                        bv_bc.rearrange("p (h d) -> p h d", h=H),
                        mybir.AluOpType.add,
                    )
                nc.vector.tensor_copy(
                    v_sb[:, :, :, DH],
                    ones60.rearrange("p (t h) -> p t h", t=NT),
                )

                if DEBUG and b == 0:
                    nc.sync.dma_start(dbg["v"][:], v_sb[:, :, :, :])

                # ---- diagonal logits for all heads: Zdiag[h, j] = q_j . k_j
                psz = ppN.tile([12, 2, 512], F32, tag="ppN")
                for f in range(NF):
                    qkel = ptrans.tile([128, NPAD], DT, tag="qkel")
                    nc.vector.tensor_tensor(
                        qkel[:, :],
                        qk_sb[:, f, :],
                        qk_sb[:, NF + f, :],
                        mybir.AluOpType.mult,
                    )
                    for s in range(2):
                        nc.tensor.matmul(
                            psz[:, s, 0:320],
                            onehd_sb[:, f, :],
                            qkel[:, s * 320 : s * 320 + 320],
                            start=(f == 0),
                            stop=(f == NF - 1),
                        )
                d_all = singles.tile([H, NPAD], F32, tag="d_all")
                nc.scalar.activation(
                    d_all.rearrange("h (s q) -> h s q", s=2),
                    psz[:, :, 0:320],
                    mybir.ActivationFunctionType.Exp, scale=SCALE,
                )
                d_t = psmall.tile([128, NT, H], F32, tag="d_t")
                for t in range(NT):
                    pst_f = ppT.tile([128, 128], F32, tag="ppT")
                    nc.tensor.transpose(
                        pst_f[:, 0:H],
                        d_all[:, t * 128 : (t + 1) * 128],
                        ident_f32[0:H, 0:H],
                    )
                    nc.vector.tensor_copy(d_t[:, t, :], pst_f[:, 0:H])

                if DEBUG and b == 0:
                    nc.sync.dma_start(dbg["dt"][:], d_t[:, :, :])

                # ---- attention, head pairs (even: PE rows 0:64, odd: 64:128) ----
                attn_sb = pb.tile([128, NT, C], DT, tag="attn_sb")
                for hp in range(H // 2):
                    he, ho = 2 * hp, 2 * hp + 1
                    nump = ppN.tile([DH + 1, 2, 512], F32, tag="ppN")
                    e_odd = pe_pool.tile([128, NT, NPAD], DT, tag="e_odd")
                    for t in range(NT):
                        st_e = ppA.tile([128, 2, 512], F32, tag="ppA")
                        st_o = ppA.tile([128, 2, 512], F32, tag="ppA")
                        for s in range(2):
                            nc.tensor.matmul(
                                st_e[:, s, 0:320],
                                qk_sb[0:DH, NF + hp, t * 128 : (t + 1) * 128],
                                qk_sb[0:DH, hp, s * 320 : s * 320 + 320],
                                start=True,
                                stop=True,
                            )
                            nc.tensor.matmul(
                                st_o[:, s, 0:320],
                                qk_sb[DH:128, NF + hp, t * 128 : (t + 1) * 128],
                                qk_sb[DH:128, hp, s * 320 : s * 320 + 320],
                                start=True,
                                stop=True,
                            )
                        e = pe_pool.tile([128, NPAD], DT, tag="e")
                        nc.scalar.activation(
                            e.rearrange("p (s q) -> p s q", s=2),
                            st_e[:, :, 0:320],
                            mybir.ActivationFunctionType.Exp,
                            bias=mb_sb[:, t : t + 1], scale=SCALE,
                        )
                        nc.scalar.activation(
                            e_odd[:, t, :].rearrange("p (s q) -> p s q", s=2),
                            st_o[:, :, 0:320],
                            mybir.ActivationFunctionType.Exp,
                            bias=mb_sb[:, t : t + 1], scale=SCALE,
                        )
                        nc.vector.scalar_tensor_tensor(
                            out=e[:, t * 128 : (t + 1) * 128],
                            in0=ident,
                            scalar=d_t[:, t, he : he + 1],
                            in1=e[:, t * 128 : (t + 1) * 128],
                            op0=mybir.AluOpType.mult,
                            op1=mybir.AluOpType.max,
                        )
                        nc.vector.scalar_tensor_tensor(
                            out=e_odd[:, t, t * 128 : (t + 1) * 128],
                            in0=ident,
                            scalar=d_t[:, t, ho : ho + 1],
                            in1=e_odd[:, t, t * 128 : (t + 1) * 128],
                            op0=mybir.AluOpType.mult,
                            op1=mybir.AluOpType.max,
                        )
                        for s in range(2):
                            nc.tensor.matmul(
                                nump[:, s, 0:320],
                                v_sb[:, t, he, :],
                                e[:, s * 320 : s * 320 + 320],
                                start=(t == 0),
                                stop=(t == NT - 1),
                            )
                    for parity, h in ((0, he), (1, ho)):
                        if parity == 1:
                            nump = ppN.tile([DH + 1, 2, 512], F32, tag="ppN")
                            for t in range(NT):
                                for s in range(2):
                                    nc.tensor.matmul(
                                        nump[:, s, 0:320],
                                        v_sb[:, t, h, :],
                                        e_odd[:, t, s * 320 : s * 320 + 320],
                                        start=(t == 0),
                                        stop=(t == NT - 1),
                                    )
                        num_sb = pnum.tile([DH + 1, NPAD], F32, tag="num_sb")
                        nc.scalar.activation(
                            num_sb.rearrange("p (s q) -> p s q", s=2),
                            nump[:, :, 0:320],
                            mybir.ActivationFunctionType.Identity,
                        )
                        # transpose to token-major (one psum bank), normalize
                        nt_all = ppT.tile([128, NT, DH + 1], F32, tag="ppT")
                        for t in range(NT):
                            nc.tensor.transpose(
                                nt_all[:, t, :],
                                num_sb[:, t * 128 : (t + 1) * 128],
                                ident_f32[0 : DH + 1, 0 : DH + 1],
                            )
                        r = psmall.tile([128, NT], F32, tag="r")
                        nc.vector.reciprocal(r[:, :], nt_all[:, :, DH])
                        nc.vector.tensor_tensor(
                            attn_sb.rearrange("p t (g d) -> p t g d", g=H)[:, :, h, :],
                            nt_all[:, :, 0:DH],
                            r[:, :, None].to_broadcast([128, NT, DH]),
                            mybir.AluOpType.mult,
                        )

                # ---- proj (transpose attn per 128x128 tile on the fly) ----
                for t in range(NT):
                    pso = ppA.tile([128, 2, 512], F32, tag="ppA")
                    for f in range(NF):
                        pst = ppT.tile([128, 128], DT, tag="ppT")
                        nc.tensor.transpose(
                            pst[:, 0:128],
                            attn_sb[:, t, f * 128 : (f + 1) * 128],
                            ident,
                        )
                        aT = ptrans.tile([128, 128], DT, tag="aT")
                        nc.vector.tensor_copy(aT[:, :], pst[:, 0:128])
                        for n0, n1 in ((0, 512), (512, 768)):
                            nc.tensor.matmul(
                                pso.rearrange("p s q -> p (s q)")[:, n0:n1],
                                aT[:, :],
                                wproj_sb[:, f, n0:n1],
                                start=(f == 0),
                                stop=(f == NF - 1),
                            )
                    o_sb = pout.tile([128, C], F32, tag="o_sb")
                    nc.vector.tensor_tensor(
                        o_sb[:, :],
                        pso.rearrange("p s q -> p (s q)")[:, 0:C],
                        bproj_bc[:, :],
                        mybir.AluOpType.add,
                    )
                    rows = 128 if t < NT - 1 else N - 4 * 128
                    nc.sync.dma_start(
                        out_d[b * N + t * 128 : b * N + t * 128 + rows, :],
                        o_sb[0:rows, :],
                    )
    nc.finalize()
    return nc


_NC_CACHE = None


def _get_nc():
    global _NC_CACHE
    if _NC_CACHE is None:
        _NC_CACHE = build_kernel()
    return _NC_CACHE


def _make_in_maps(x, policy, Wqkv, bqkv, Wproj, bproj):
    x = np.ascontiguousarray(np.asarray(x, dtype=np.float32))
    policy = np.asarray(policy, dtype=np.float32).reshape(B, N)
    Wqkv = np.ascontiguousarray(np.asarray(Wqkv, dtype=np.float32))
    bqkv = np.ascontiguousarray(np.asarray(bqkv, dtype=np.float32))
    Wproj = np.ascontiguousarray(np.asarray(Wproj, dtype=np.float32))
    bproj = np.ascontiguousarray(np.asarray(bproj, dtype=np.float32))

    npdt = ml_dtypes.bfloat16 if MMDT_NAME == "bf16" else np.float32
    xpad = np.zeros((B, NPAD, C), dtype=np.float32)
    xpad[:, :N, :] = x
    xpad = xpad.astype(npdt)
    Wqkv = Wqkv.astype(npdt)
    Wproj = Wproj.astype(npdt)
    maskb = np.full((B, NPAD), MASK_NEG, dtype=np.float32)
    maskb[:, :N] = np.where(policy > 0.5, 0.0, MASK_NEG)

    ones_hd = np.zeros((C, H), dtype=np.float32)
    for h in range(H):
        ones_hd[h * DH : (h + 1) * DH, h] = 1.0
    ones_hd = ones_hd.astype(npdt)

    in_maps = []
    for c in range(NCORES):
        b0 = c * BB
        in_maps.append(
            {
                "x": xpad[b0 : b0 + BB].reshape(BB * NPAD, C),
                "maskb": maskb[b0 : b0 + BB],
                "wqkv": Wqkv,
                "wproj": Wproj,
                "bqkv": bqkv,
                "bproj": bproj,
                "ones_hd": ones_hd,
            }
        )
    return in_maps


def run(inputs, trace=False):
    """Run on hardware; returns (output [B,N,C], BassKernelResults)."""
    from concourse.bass_utils import run_bass_kernel_spmd

    nc = _get_nc()
    in_maps = _make_in_maps(**inputs)
    res = run_bass_kernel_spmd(
        nc, in_maps, core_ids=list(range(NCORES)), trace=trace
    )
    out = np.empty((B, N, C), dtype=np.float32)
    for c in range(NCORES):
        out[c * BB : (c + 1) * BB] = res.results[c]["out"].reshape(BB, N, C)
    return out, res


def kernel(x, policy, Wqkv, bqkv, Wproj, bproj):
    out, _ = run(
        dict(x=x, policy=policy, Wqkv=Wqkv, bqkv=bqkv, Wproj=Wproj, bproj=bproj)
    )
    return out


# revision 37
# speedup vs baseline: 1.0296x; 1.0296x over previous
"""Policy-masked multi-head attention block (ViT-style) on 8 TRN2 NeuronCores.

Sharding: data-parallel over batch. B=16 -> 2 batches per core, no collectives.

Math (per batch, matches reference up to O(1e-7)):
  qkv = x @ Wqkv + bqkv ; q,k,v per head (Dh=64)
  E[j,q] = exp(SCALE*z[j,q] + mb_j)   (z = k_j . q_q raw logits; mb_j = -1e4 for
                                       dropped/padded keys, 0 otherwise)
  E[q,q] = exp(SCALE*z[q,q])          (diagonal always kept)
  num[d,q] = sum_j v[j,d]*E[j,q] + (EPS/N)*sum_j v[j,d]
  den[q]   = sum_j E[j,q] + EPS
  attn_out[q,d] = num[d,q]/den[q] ;  out = attn_out @ Wproj + bproj

No max-subtraction: |SCALE*z| is O(1) for this input distribution, and the
reference's max-shift cancels in the ratio except for EPS terms (~1e-8 rel).

Layout: attention is computed keys-on-partitions / queries-on-free so that
 - the policy mask is a per-partition activation bias (free on ACT),
 - the key-sum (softmax denominator) rides the AV matmul as a ones column of V,
 - E is consumed directly as the moving operand of the AV matmul (no P matrix
   transpose).
"""

import os
import ml_dtypes
import numpy as np

import concourse.bass as bass
import concourse.bacc as bacc
import concourse.mybir as mybir
import concourse.tile as tile
from concourse.masks import make_identity

# problem constants (hardcoded per contract)
B = 16
N = 577
C = 768
H = 12
DH = 64
SCALE = DH ** -0.5
EPS = 1e-6
NCORES = 8
BB = B // NCORES          # batches per core
NPAD = 640                # tokens padded to 5*128
NT = NPAD // 128          # 5 token tiles
NF = C // 128             # 6 feature tiles
F32 = mybir.dt.float32
F32R = mybir.dt.float32r
BF16 = mybir.dt.bfloat16
MMDT_NAME = os.environ.get("KERNEL_MMDT", "bf16")
DT = BF16 if MMDT_NAME == "bf16" else F32R

MASK_NEG = -1.0e4
NQ1 = 320              # query region sizes: [0:320), [320:577)
NQ2 = N - 320          # 257


DEBUG = bool(int(os.environ.get("KERNEL_DEBUG", "0")))


def build_kernel():
    nc = bacc.Bacc()
    dbg = {}
    if DEBUG:
        dbg["qk"] = nc.declare_dram_parameter("dbg_qk", [128, 12, NPAD], F32, isOutput=True)
        dbg["v"] = nc.declare_dram_parameter("dbg_v", [128, NT, H, DH + 1], F32, isOutput=True)
        dbg["dt"] = nc.declare_dram_parameter("dbg_dt", [128, NT, H], F32, isOutput=True)
        dbg["e"] = nc.declare_dram_parameter("dbg_e", [NT, 128, NPAD], F32, isOutput=True)
        dbg["num"] = nc.declare_dram_parameter("dbg_num", [DH + 1, NPAD], F32, isOutput=True)
        dbg["attn"] = nc.declare_dram_parameter("dbg_attn", [128, NT, C], F32, isOutput=True)

    x_d = nc.declare_dram_parameter("x", [BB * NPAD, C], DT, isOutput=False)
    mb_d = nc.declare_dram_parameter("maskb", [BB, NPAD], F32, isOutput=False)
    wqkv_d = nc.declare_dram_parameter("wqkv", [C, 3 * C], DT, isOutput=False)
    wproj_d = nc.declare_dram_parameter("wproj", [C, C], DT, isOutput=False)
    bqkv_d = nc.declare_dram_parameter("bqkv", [3 * C], F32, isOutput=False)
    bproj_d = nc.declare_dram_parameter("bproj", [C], F32, isOutput=False)
    onehd_d = nc.declare_dram_parameter("ones_hd", [C, H], DT, isOutput=False)
    out_d = nc.declare_dram_parameter("out", [BB * N, C], F32, isOutput=True)

    with tile.TileContext(nc) as tc:
        with (
            tc.tile_pool(name="singles", bufs=1) as singles,
            tc.tile_pool(name="pbatch", bufs=1) as pb,
            tc.tile_pool(name="pqk", bufs=2) as pqk,
            tc.tile_pool(name="pe", bufs=3) as pe_pool,
            tc.tile_pool(name="pnum", bufs=2) as pnum,
            tc.tile_pool(name="psmall", bufs=4) as psmall,
            tc.tile_pool(name="ptrans", bufs=2) as ptrans,
            tc.tile_pool(name="pout", bufs=2) as pout,
            tc.tile_pool(name="ppA", bufs=2, space="PSUM") as ppA,
            tc.tile_pool(name="ppT", bufs=2, space="PSUM") as ppT,
            tc.tile_pool(name="ppN", bufs=1, space="PSUM") as ppN,
        ):
            # ---- constants ----
            wqkv_sb = singles.tile([128, NF, 3 * C], DT)
            for f in range(NF):
                nc.sync.dma_start(
                    wqkv_sb[:, f, :],
                    wqkv_d.rearrange("(f p) m -> f p m", p=128)[f],
                )
            wproj_sb = singles.tile([128, NF, C], DT)
            for f in range(NF):
                nc.scalar.dma_start(
                    wproj_sb[:, f, :],
                    wproj_d.rearrange("(f p) m -> f p m", p=128)[f],
                )
            bqkv_sb = singles.tile([128, 2 * NF], F32)  # q,k feature bias chunks
            nc.scalar.dma_start(
                bqkv_sb[:, :],
                bqkv_d[0 : 2 * C].rearrange("(m p) -> p m", p=128),
            )
            # v-bias and proj-bias broadcast across all 128 partitions
            bv_bc = singles.tile([128, C], F32)
            nc.gpsimd.dma_start(
                out=bv_bc[:, :], in_=bqkv_d[2 * C : 3 * C].partition_broadcast(128)
            )
            bproj_bc = singles.tile([128, C], F32)
            nc.gpsimd.dma_start(
                out=bproj_bc[:, :], in_=bproj_d[:].partition_broadcast(128)
            )
            onehd_sb = singles.tile([128, NF, H], DT)
            nc.scalar.dma_start(
                onehd_sb[:, :, :],
                onehd_d.rearrange("(f p) h -> p f h", p=128),
            )
            ident_f32 = singles.tile([128, 128], F32)
            make_identity(nc, ident_f32)
            ident = singles.tile([128, 128], DT)
            nc.vector.tensor_copy(ident[:, :], ident_f32[:, :])
            ones60 = singles.tile([128, NT * H], F32)
            nc.vector.memset(ones60, 1.0)

            for b in range(BB):
                # ---- load x (token-major chunks), transpose to xT ----
                mb_sb = psmall.tile([128, NT], F32, tag="mb_sb")
                nc.sync.dma_start(
                    mb_sb[:, :], mb_d[b].rearrange("(t p) -> p t", p=128)
                )
                xT = pb.tile([128, NF, NPAD], DT, tag="xT")
                for f in range(NF):
                    nc.sync.dma_start_transpose(
                        xT[:, f, :],
                        x_d[b * NPAD : (b + 1) * NPAD,
                            f * 128 : (f + 1) * 128],
                    )

                # ---- QKV: q,k sections feature-major ----
                # qk_sb[p, m, :]: m in 0..5 -> q features, 6..11 -> k features
                qk_sb = pqk.tile([128, 2 * NF, NPAD], DT, tag="qk_sb")
                for m in range(2 * NF):
                    ps = ppA.tile([128, 2, 512], F32, tag="ppA")
                    for f in range(NF):
                        for s in range(2):
                            nc.tensor.matmul(
                                ps[:, s, 0:320],
                                wqkv_sb[:, f, m * 128 : (m + 1) * 128],
                                xT[:, f, s * 320 : s * 320 + 320],
                                start=(f == 0),
                                stop=(f == NF - 1),
                            )
                    nc.vector.tensor_scalar_add(
                        qk_sb[:, m, :].rearrange("p (s q) -> p s q", s=2),
                        ps[:, :, 0:320],
                        bqkv_sb[:, m : m + 1],
                    )

                if DEBUG and b == 0:
                    nc.sync.dma_start(dbg["qk"][:], qk_sb[:, :, :])

                # ---- QKV: v section token-major, per-head layout with ones col
                v_sb = pb.tile([128, NT, H, DH + 1], DT, tag="v_sb")
                for t in range(NT):
                    ps = ppA.tile([128, 2, 512], F32, tag="ppA")
                    for f in range(NF):
                        for n0, n1 in ((0, 512), (512, 768)):
                            nc.tensor.matmul(
                                ps.rearrange("p s q -> p (s q)")[:, n0:n1],
                                xT[:, f, t * 128 : (t + 1) * 128],
                                wqkv_sb[:, f, 2 * C + n0 : 2 * C + n1],
                                start=(f == 0),
                                stop=(f == NF - 1),
                            )
                    nc.vector.tensor_tensor(
                        v_sb[:, t, :, 0:DH],
                        ps.rearrange("p s q -> p (s q)")[:, 0:C].rearrange(
                            "p (h d) -> p h d", h=H
                        ),
                        bv_bc.rearrange("p (h d) -> p h d", h=H),
                        mybir.AluOpType.add,
                    )
                nc.vector.tensor_copy(
                    v_sb[:, :, :, DH],
                    ones60.rearrange("p (t h) -> p t h", t=NT),
                )

                if DEBUG and b == 0:
                    nc.sync.dma_start(dbg["v"][:], v_sb[:, :, :, :])

                # ---- diagonal logits for all heads: Zdiag[h, j] = q_j . k_j
                psz = ppN.tile([12, 2, 512], F32, tag="ppN")
                for f in range(NF):
                    qkel = ptrans.tile([128, NPAD], DT, tag="qkel")
                    nc.gpsimd.tensor_tensor(
                        qkel[:, :],
                        qk_sb[:, f, :],
                        qk_sb[:, NF + f, :],
                        mybir.AluOpType.mult,
                    )
                    for s in range(2):
                        nc.tensor.matmul(
                            psz[:, s, 0:320],
                            onehd_sb[:, f, :],
                            qkel[:, s * 320 : s * 320 + 320],
                            start=(f == 0),
                            stop=(f == NF - 1),
                        )
                d_all = singles.tile([H, NPAD], F32, tag="d_all")
                nc.scalar.activation(
                    d_all.rearrange("h (s q) -> h s q", s=2),
                    psz[:, :, 0:320],
                    mybir.ActivationFunctionType.Exp, scale=SCALE,
                )
                d_t = psmall.tile([128, NT, H], F32, tag="d_t")
                for t in range(NT):
                    pst_f = ppT.tile([128, 128], F32, tag="ppT")
                    nc.tensor.transpose(
                        pst_f[:, 0:H],
                        d_all[:, t * 128 : (t + 1) * 128],
                        ident_f32[0:H, 0:H],
                    )
                    nc.vector.tensor_copy(d_t[:, t, :], pst_f[:, 0:H])

                if DEBUG and b == 0:
                    nc.sync.dma_start(dbg["dt"][:], d_t[:, :, :])

                # ---- attention, head pairs (even: PE rows 0:64, odd: 64:128) ----
                attn_sb = pb.tile([128, NT, C], DT, tag="attn_sb")
                for hp in range(H // 2):
                    he, ho = 2 * hp, 2 * hp + 1
                    nump = ppN.tile([DH + 1, 2, 512], F32, tag="ppN")
                    e_odd = pe_pool.tile([128, NT, NPAD], DT, tag="e_odd")
                    for t in range(NT):
                        st_e = ppA.tile([128, 2, 512], F32, tag="ppA")
                        st_o = ppA.tile([128, 2, 512], F32, tag="ppA")
                        for s in range(2):
                            nc.tensor.matmul(
                                st_e[:, s, 0:320],
                                qk_sb[0:DH, NF + hp, t * 128 : (t + 1) * 128],
                                qk_sb[0:DH, hp, s * 320 : s * 320 + 320],
                                start=True,
                                stop=True,
                            )
                            nc.tensor.matmul(
                                st_o[:, s, 0:320],
                                qk_sb[DH:128, NF + hp, t * 128 : (t + 1) * 128],
                                qk_sb[DH:128, hp, s * 320 : s * 320 + 320],
                                start=True,
                                stop=True,
                            )
                        e = pe_pool.tile([128, NPAD], DT, tag="e", bufs=5)
                        nc.scalar.activation(
                            e.rearrange("p (s q) -> p s q", s=2),
                            st_e[:, :, 0:320],
                            mybir.ActivationFunctionType.Exp,
                            bias=mb_sb[:, t : t + 1], scale=SCALE,
                        )
                        nc.scalar.activation(
                            e_odd[:, t, :].rearrange("p (s q) -> p s q", s=2),
                            st_o[:, :, 0:320],
                            mybir.ActivationFunctionType.Exp,
                            bias=mb_sb[:, t : t + 1], scale=SCALE,
                        )
                        nc.vector.scalar_tensor_tensor(
                            out=e[:, t * 128 : (t + 1) * 128],
                            in0=ident,
                            scalar=d_t[:, t, he : he + 1],
                            in1=e[:, t * 128 : (t + 1) * 128],
                            op0=mybir.AluOpType.mult,
                            op1=mybir.AluOpType.max,
                        )
                        nc.vector.scalar_tensor_tensor(
                            out=e_odd[:, t, t * 128 : (t + 1) * 128],
                            in0=ident,
                            scalar=d_t[:, t, ho : ho + 1],
                            in1=e_odd[:, t, t * 128 : (t + 1) * 128],
                            op0=mybir.AluOpType.mult,
                            op1=mybir.AluOpType.max,
                        )
                        for s in range(2):
                            nc.tensor.matmul(
                                nump[:, s, 0:320],
                                v_sb[:, t, he, :],
                                e[:, s * 320 : s * 320 + 320],
                                start=(t == 0),
                                stop=(t == NT - 1),
                            )
                    for parity, h in ((0, he), (1, ho)):
                        if parity == 1:
                            nump = ppN.tile([DH + 1, 2, 512], F32, tag="ppN")
                            for t in range(NT):
                                for s in range(2):
                                    nc.tensor.matmul(
                                        nump[:, s, 0:320],
                                        v_sb[:, t, h, :],
                                        e_odd[:, t, s * 320 : s * 320 + 320],
                                        start=(t == 0),
                                        stop=(t == NT - 1),
                                    )
                        num_sb = pnum.tile([DH + 1, NPAD], F32, tag="num_sb")
                        nc.scalar.activation(
                            num_sb.rearrange("p (s q) -> p s q", s=2),
                            nump[:, :, 0:320],
                            mybir.ActivationFunctionType.Identity,
                        )
                        # transpose to token-major (one psum bank), normalize
                        nt_all = ppT.tile([128, NT, DH + 1], F32, tag="ppT")
                        for t in range(NT):
                            nc.tensor.transpose(
                                nt_all[:, t, :],
                                num_sb[:, t * 128 : (t + 1) * 128],
                                ident_f32[0 : DH + 1, 0 : DH + 1],
                            )
                        r = psmall.tile([128, NT], F32, tag="r")
                        nc.vector.reciprocal(r[:, :], nt_all[:, :, DH])
                        nc.vector.tensor_tensor(
                            attn_sb.rearrange("p t (g d) -> p t g d", g=H)[:, :, h, :],
                            nt_all[:, :, 0:DH],
                            r[:, :, None].to_broadcast([128, NT, DH]),
                            mybir.AluOpType.mult,
                        )

                # ---- proj (transpose attn per 128x128 tile on the fly) ----
                for t in range(NT):
                    pso = ppA.tile([128, 2, 512], F32, tag="ppA")
                    for f in range(NF):
                        pst = ppT.tile([128, 128], DT, tag="ppT")
                        nc.tensor.transpose(
                            pst[:, 0:128],
                            attn_sb[:, t, f * 128 : (f + 1) * 128],
                            ident,
                        )
                        aT = ptrans.tile([128, 128], DT, tag="aT")
                        nc.vector.tensor_copy(aT[:, :], pst[:, 0:128])
                        for n0, n1 in ((0, 512), (512, 768)):
                            nc.tensor.matmul(
                                pso.rearrange("p s q -> p (s q)")[:, n0:n1],
                                aT[:, :],
                                wproj_sb[:, f, n0:n1],
                                start=(f == 0),
                                stop=(f == NF - 1),
                            )
                    o_sb = pout.tile([128, C], F32, tag="o_sb")
                    nc.vector.tensor_tensor(
                        o_sb[:, :],
                        pso.rearrange("p s q -> p (s q)")[:, 0:C],
                        bproj_bc[:, :],
                        mybir.AluOpType.add,
                    )
                    rows = 128 if t < NT - 1 else N - 4 * 128
                    nc.sync.dma_start(
                        out_d[b * N + t * 128 : b * N + t * 128 + rows, :],
                        o_sb[0:rows, :],
                    )
    nc.finalize()
    return nc


_NC_CACHE = None


def _get_nc():
    global _NC_CACHE
    if _NC_CACHE is None:
        _NC_CACHE = build_kernel()
    return _NC_CACHE


def _make_in_maps(x, policy, Wqkv, bqkv, Wproj, bproj):
    x = np.ascontiguousarray(np.asarray(x, dtype=np.float32))
    policy = np.asarray(policy, dtype=np.float32).reshape(B, N)
    Wqkv = np.ascontiguousarray(np.asarray(Wqkv, dtype=np.float32))
    bqkv = np.ascontiguousarray(np.asarray(bqkv, dtype=np.float32))
    Wproj = np.ascontiguousarray(np.asarray(Wproj, dtype=np.float32))
    bproj = np.ascontiguousarray(np.asarray(bproj, dtype=np.float32))

    npdt = ml_dtypes.bfloat16 if MMDT_NAME == "bf16" else np.float32
    xpad = np.zeros((B, NPAD, C), dtype=np.float32)
    xpad[:, :N, :] = x
    xpad = xpad.astype(npdt)
    Wqkv = Wqkv.astype(npdt)
    Wproj = Wproj.astype(npdt)
    maskb = np.full((B, NPAD), MASK_NEG, dtype=np.float32)
    maskb[:, :N] = np.where(policy > 0.5, 0.0, MASK_NEG)

    ones_hd = np.zeros((C, H), dtype=np.float32)
    for h in range(H):
        ones_hd[h * DH : (h + 1) * DH, h] = 1.0
    ones_hd = ones_hd.astype(npdt)

    in_maps = []
    for c in range(NCORES):
        b0 = c * BB
        in_maps.append(
            {
                "x": xpad[b0 : b0 + BB].reshape(BB * NPAD, C),
                "maskb": maskb[b0 : b0 + BB],
                "wqkv": Wqkv,
                "wproj": Wproj,
                "bqkv": bqkv,
                "bproj": bproj,
                "ones_hd": ones_hd,
            }
        )
    return in_maps


def run(inputs, trace=False):
    """Run on hardware; returns (output [B,N,C], BassKernelResults)."""
    from concourse.bass_utils import run_bass_kernel_spmd

    nc = _get_nc()
    in_maps = _make_in_maps(**inputs)
    res = run_bass_kernel_spmd(
        nc, in_maps, core_ids=list(range(NCORES)), trace=trace
    )
    out = np.empty((B, N, C), dtype=np.float32)
    for c in range(NCORES):
        out[c * BB : (c + 1) * BB] = res.results[c]["out"].reshape(BB, N, C)
    return out, res


def kernel(x, policy, Wqkv, bqkv, Wproj, bproj):
    out, _ = run(
        dict(x=x, policy=policy, Wqkv=Wqkv, bqkv=bqkv, Wproj=Wproj, bproj=bproj)
    )
    return out
